# revision 24
# baseline (speedup 1.0000x reference)
"""Trainium2 Bass kernel for RoPE multi-head attention (B=2, T=2048, D=1024, H=16).

Sharding: tensor-parallel over heads (2 heads / core). Pipeline structure:
  - X^T loaded per 512-token chunk (64 x 128KB DMAs so chunks complete in
    order); Q/K projection + RoPE and V^T production per chunk keep the PE
    dense from ~3us in (HAM warm).
  - Attention per (batch, 1024-tok group): scores for both heads run
    CONCURRENTLY via PE row-tiling (tile_position (0,0)/(64,0), K=64 each);
    PV via col-tiling (both heads into one PSUM bank); softmax denominators
    via col-tiled ones-stationary matmuls; exp is one 2-head ACT per
    (keytile, chunk) with element-granular causal pruning.
  - RoPE tables: one range reduction; signed sin in one ACT via a +-1
    per-partition scale (sin is odd); cos = 1 - 2 sin^2(x/2).
  - AllToAll head->token resharding chunked into 4 x 256KB collectives, each
    fired as soon as its 1024-tok group finishes, overlapped with later
    groups' attention. Output projection per chunk in transposed form
    (out^T = sum_s ao_s^T @ WoT_s, 16 N=512 matmuls).
"""

import sys
from collections import deque
from contextlib import ExitStack

for _p in ("/opt/trn_rl_repo",):
    if _p not in sys.path:
        sys.path.append(_p)

import numpy as np
import ml_dtypes

import concourse.bacc as bacc
import concourse.bass as bass
import concourse.mybir as mybir
import concourse.tile as tile
from concourse.bass_utils import run_bass_kernel_spmd

# Problem constants (hardcoded per harness contract).
B, T, D, H, DH = 2, 2048, 1024, 16, 64
NCORES = 8
TOK = B * T                # 4096 flattened tokens
THETA = 10000.0
SCALE = 1.0 / 8.0          # 1/sqrt(DH)
KS = D // 128              # 8 contraction slices
NCH = TOK // 512           # 8 x 512-tok chunks total
NG = 4                     # 4 x 1024-tok a2a groups
TPC = TOK // NCORES        # 512 tokens per core for the output projection

BF16 = mybir.dt.bfloat16
F32 = mybir.dt.float32
FP8 = mybir.dt.float8e4
WSCALE = 64.0  # fp8 weight scaling; compensated in exp scale / ones33


def build_nc(skip_collective=False, dummy=False):
    nc = bacc.Bacc(
        "TRN2",
        target_bir_lowering=False,
        debug=False,
        num_devices=NCORES,
    )

    # ---- kernel I/O ----
    xt_d = nc.dram_tensor("xt", [D, TOK], BF16, kind="ExternalInput")
    wqt_d = nc.dram_tensor("wqt", [D, 128], BF16, kind="ExternalInput")
    wkt_d = nc.dram_tensor("wkt", [D, 128], BF16, kind="ExternalInput")
    wvt_d = nc.dram_tensor("wvt", [D, 128], BF16, kind="ExternalInput")
    wot_d = nc.dram_tensor("wot", [D, D], BF16, kind="ExternalInput")
    posf_d = nc.dram_tensor("posf", [1, T], F32, kind="ExternalInput")
    # out^T rows: chunk g covers flat toks 1024*g + 128*core + [0:128)
    out_d = nc.dram_tensor("out", [TPC, D], F32, kind="ExternalOutput")

    # ---- compile-time constants ----
    inv_freq = (1.0 / THETA ** (np.arange(DH // 2, dtype=np.float64) / (DH // 2)))
    f = inv_freq.astype(np.float32)
    # rows follow the 16-stripe rope layout: [f0:16, f0:16, f16:32, f16:32]/head
    invf4 = np.concatenate([f[0:16], f[0:16], f[16:32], f[16:32]] * 2)[None, :]
    invf_d = nc.inline_tensor(invf4, "invf")
    # tri[j, i] = 1 iff key j <= tok i; duplicated for the 2-head 3D AP.
    tri = np.triu(np.ones((128, 128), np.float32))
    tri2 = np.concatenate([tri, tri], axis=1).astype(ml_dtypes.bfloat16)
    tri2_d = nc.inline_tensor(tri2, "tri2")

    if dummy:
        with tile.TileContext(nc) as tc, ExitStack() as ctx:
            sp = ctx.enter_context(tc.tile_pool(name="sp", bufs=1))
            z = sp.tile([128, 16], F32, tag="z")
            nc.vector.memset(z, 0.0)
            nc.sync.dma_start(out=out_d[0:128, 0:16], in_=z)
        nc.compile()
        return nc

    with tile.TileContext(nc) as tc, ExitStack() as ctx:
        singles = ctx.enter_context(tc.tile_pool(name="singles", bufs=1))
        tmp = ctx.enter_context(tc.tile_pool(name="tmp", bufs=2))
        ppool = ctx.enter_context(tc.tile_pool(name="ppool", bufs=4))
        apool = ctx.enter_context(tc.tile_pool(name="apool", bufs=2))
        opool = ctx.enter_context(tc.tile_pool(name="opool", bufs=2))
        # PSUM: psA 2x[128,2,512]=4 banks (scores/qkv/bc/outproj),
        # psB 3x[128,512]=3 (pv accumulators + V/angle transients),
        # psC 1x[128,512]=1 (all 4 rowsum accumulators of a group) -> 8.
        psA = ctx.enter_context(tc.tile_pool(name="psA", bufs=2, space="PSUM"))
        psB = ctx.enter_context(tc.tile_pool(name="psB", bufs=3, space="PSUM"))
        psC = ctx.enter_context(tc.tile_pool(name="psC", bufs=1, space="PSUM"))
        dpool = ctx.enter_context(tc.tile_pool(name="dram", bufs=1, space="DRAM"))

        # ---- persistent SBUF tensors; small inputs first ----
        posf = singles.tile([1, T], F32, tag="posf")
        nc.sync.dma_start(out=posf, in_=posf_d.ap())
        invf = singles.tile([1, 128], F32, tag="invf")
        nc.sync.dma_start(out=invf, in_=invf_d.ap())
        tri_sb = singles.tile([128, 2, 128], BF16, tag="tri")
        nc.sync.dma_start(
            out=tri_sb, in_=tri2_d.ap().rearrange("p (h f) -> p h f", h=2)
        )
        wq = singles.tile([128, KS, 128], BF16, tag="wq")
        nc.sync.dma_start(out=wq, in_=wqt_d.ap().rearrange("(k p) m -> p k m", p=128))
        wk = singles.tile([128, KS, 128], BF16, tag="wk")
        nc.sync.dma_start(out=wk, in_=wkt_d.ap().rearrange("(k p) m -> p k m", p=128))
        wv = singles.tile([128, KS, 128], BF16, tag="wv")
        nc.sync.dma_start(out=wv, in_=wvt_d.ap().rearrange("(k p) m -> p k m", p=128))

        # X^T loaded per (chunk, k-slice): 64 x 128KB so chunk c completes
        # ~in order c across the DMA queues (QKV round c starts early).
        xt = singles.tile([128, KS, TOK], BF16, tag="xt")
        xt_r = xt_d.ap().rearrange("(k p) t -> p k t", p=128)
        for c in range(NCH):
            cols = slice(c * 512, (c + 1) * 512)
            for k in range(KS):
                nc.sync.dma_start(out=xt[:, k, cols], in_=xt_r[:, k, cols])
        wo = singles.tile([128, KS, D], BF16, tag="wo")
        nc.sync.dma_start(out=wo, in_=wot_d.ap().rearrange("(k p) f -> p k f", p=128))

        qt = singles.tile([128, TOK], BF16, tag="qt")     # rotated Q^T
        kt_sb = singles.tile([128, TOK], BF16, tag="kt")  # rotated K^T
        # V^T per keytile block: [keys=128 partitions, 32 blocks, 128 dims]
        vsb = singles.tile([128, B * 16, 128], BF16, tag="vsb")

        # rowsum stationary: [128, 32] ones (replicates rowsum over 32 rows)
        onesK = singles.tile([128, 32], BF16, tag="onesK")
        nc.vector.memset(onesK, 1.0)
        # bc stationary: row 0 -> out dims 0:64 (h0), row 32 -> 64:128 (h1)
        ones33 = singles.tile([33, 128], BF16, tag="ones33")
        nc.vector.memset(ones33, 0.0)
        nc.vector.memset(ones33[0:1, 0:64], 1.0)
        nc.vector.memset(ones33[32:33, 64:128], 1.0)
        # per-partition sign for the rotate-half sin table (sin is odd:
        # Sin(red * sign) = sign * sin(red))
        sign_ap = singles.tile([128, 1], F32, tag="sign")
        nc.vector.memset(sign_ap, 1.0)
        for q in range(4):
            nc.vector.memset(sign_ap[32 * q:32 * q + 16, :], -1.0)

        # cos/sin tables [128, T] bf16; rows repeat inv_freq every 32.
        cs4 = singles.tile([128, T], BF16, tag="cs4")
        sns4 = singles.tile([128, T], BF16, tag="sns4")

        # ---- RoPE tables ----
        # ang = pos * inv_freq; red = ang - 2pi*rn(ang/2pi) in [-pi,pi]
        # (magic-number round; boundary spill ~2e-4 rad is clamped by Sin's
        # profile stage). sin table = Sin(red)*sign via scale AP; cos table
        # = 1 - 2*Sin(red/2)^2.
        tau = float(2 * np.pi)
        magic = float(1.5 * 2 ** 23)
        for q4 in range(T // 512):
            sl = slice(q4 * 512, (q4 + 1) * 512)
            ps_ang = psB.tile([128, 512], F32, tag="pv", name="ps_ang")
            nc.tensor.matmul(ps_ang, invf, posf[:, sl], start=True, stop=True)
            qm = tmp.tile([128, 512], F32, tag="ra", name="qm")
            nc.vector.tensor_scalar(
                out=qm, in0=ps_ang, scalar1=1.0 / tau, scalar2=magic,
                op0=mybir.AluOpType.mult, op1=mybir.AluOpType.add,
            )
            qr = tmp.tile([128, 512], F32, tag="rb", name="qr")
            nc.vector.tensor_scalar(
                out=qr, in0=qm, scalar1=magic, scalar2=None,
                op0=mybir.AluOpType.subtract,
            )
            rd0 = tmp.tile([128, 512], F32, tag="rc", name="rd0")
            nc.vector.scalar_tensor_tensor(
                out=rd0, in0=qr, scalar=-tau, in1=ps_ang,
                op0=mybir.AluOpType.mult, op1=mybir.AluOpType.add,
            )
            red = tmp.tile([128, 512], F32, tag="rf", name="red")
            nc.vector.tensor_scalar(
                out=red, in0=rd0, scalar1=float(np.pi), scalar2=float(-np.pi),
                op0=mybir.AluOpType.min, op1=mybir.AluOpType.max,
            )
            nc.scalar.activation(
                out=sns4[:, sl], in_=red,
                func=mybir.ActivationFunctionType.Sin, scale=sign_ap,
            )
            s2 = tmp.tile([128, 512], F32, tag="rd", name="s2")
            nc.scalar.activation(
                out=s2, in_=red, func=mybir.ActivationFunctionType.Sin, scale=0.5
            )
            sq = tmp.tile([128, 512], F32, tag="re", name="sq")
            nc.scalar.activation(
                out=sq, in_=s2, func=mybir.ActivationFunctionType.Square
            )
            nc.vector.tensor_scalar(
                out=cs4[:, sl], in0=sq, scalar1=-2.0, scalar2=1.0,
                op0=mybir.AluOpType.mult, op1=mybir.AluOpType.add,
            )

        # preload the exp ACT table set so the first attention exp doesn't
        # pay the ~2.7us table switch mid-stream
        scr = singles.tile([128, 1], F32, tag="scr")
        nc.scalar.activation(
            out=scr, in_=sign_ap, func=mybir.ActivationFunctionType.Exp
        )
        scr_d = dpool.tile([128, 1], F32, tag="scr_d", name="scr_d")
        nc.sync.dma_start(out=scr_d, in_=scr)

        # ---- a2a buffers: chunk g = 1024 toks; shard i -> dest core i ----
        a2a_in = [
            dpool.tile([NCORES, 128, 128], BF16, tag=f"a2a_in{g}",
                       name=f"a2a_in{g}")
            for g in range(NG)
        ]
        a2a_out = [
            dpool.tile([NCORES, 128, 128], BF16, tag=f"a2a_out{g}",
                       name=f"a2a_out{g}")
            for g in range(NG)
        ]

        # ---- per-chunk QKV projection + RoPE + V^T production ----
        swap_mask = [(i + 16) % 32 for i in range(32)]

        def rope(src_ps, dst, cols, tcs):
            # rot = x*cos + swap16(x)*sin_signed; swap via DVE stream_shuffle
            t1 = tmp.tile([128, 512], BF16, tag="t1", name="t1")
            nc.vector.tensor_mul(t1, src_ps, cs4[:, tcs])
            xs = tmp.tile([128, 512], F32, tag="xs", name="xs")
            nc.vector.stream_shuffle(xs, src_ps, swap_mask)
            t2 = tmp.tile([128, 512], BF16, tag="t2", name="t2")
            nc.gpsimd.tensor_mul(t2, xs, sns4[:, tcs])
            nc.vector.tensor_add(dst[:, cols], t1, t2)

        def qkv_piece_1(b, c, w_sb, dst):
            cols = slice(b * T + c * 512, b * T + (c + 1) * 512)
            tcs = slice(c * 512, (c + 1) * 512)  # position within batch
            ps_qk = psA.tile([128, 2, 512], F32, tag="big", name="ps_qk")
            for k in range(KS):
                nc.tensor.matmul(
                    ps_qk[:, 0, :], w_sb[:, k, :], xt[:, k, cols],
                    start=(k == 0), stop=(k == KS - 1),
                )
            rope(ps_qk[:, 0, :], dst, cols, tcs)

        def qkv_piece_v(b, c, jh):
            # V^T for keytile blocks [2*jh, 2*jh+1] of chunk (b, c)
            ps_v = psB.tile([128, 2, 128], F32, tag="pv", name="ps_v")
            for j in range(2):
                t0 = b * T + c * 512 + (2 * jh + j) * 128
                for k in range(KS):
                    nc.tensor.matmul(
                        ps_v[:, j, :],
                        xt[:, k, t0:t0 + 128],
                        wv[:, k, :],
                        start=(j == 0 and k == 0),
                        stop=(j == 1 and k == KS - 1),
                        skip_group_check=True,
                    )
            blk0 = b * 16 + c * 4 + 2 * jh
            nc.vector.tensor_copy(out=vsb[:, blk0:blk0 + 2, :], in_=ps_v)

        def qkv_round_fillers(b, c):
            return [
                lambda b=b, c=c: qkv_piece_1(b, c, wk, kt_sb),
                lambda b=b, c=c: qkv_piece_v(b, c, 0),
                lambda b=b, c=c: qkv_piece_1(b, c, wq, qt),
                lambda b=b, c=c: qkv_piece_v(b, c, 1),
            ]

        # ---- output projection for a2a chunk g (out^T form) ----
        def outproj(g):
            ao = opool.tile([128, KS, 128], BF16, tag="ao", name="ao")
            for s in range(KS):
                nc.sync.dma_start(out=ao[:, s, :], in_=a2a_out[g][s, :, :])
            ps_o = psA.tile([128, 2, 512], F32, tag="big", name="ps_o")
            for s in range(KS):
                nc.tensor.matmul(
                    ps_o[:, 0, :], ao[:, s, :], wo[:, s, 0:512],
                    start=(s == 0), stop=(s == KS - 1),
                )
                nc.tensor.matmul(
                    ps_o[:, 1, :], ao[:, s, :], wo[:, s, 512:1024],
                    start=(s == 0), stop=(s == KS - 1),
                )
            osb = opool.tile([128, 2, 512], F32, tag="osb", name="osb")
            nc.vector.tensor_copy(out=osb, in_=ps_o)
            nc.sync.dma_start(
                out=out_d.ap()[g * 128:(g + 1) * 128, :].rearrange(
                    "p (a f) -> p a f", a=2
                ),
                in_=osb,
            )

        # ---- attention group: (batch, c2) covers toks c2*1024..+1024 ----
        # fillers: independent PE jobs interleaved into the kt loop so the
        # PE stream stays dense (HAM warm) while ScalarE streams the exps.
        def attn_group(b, c2, fillers, defer_tail=False):
            g = 2 * b + c2
            pv = {}
            for cl in range(2):
                pv[cl] = psB.tile([128, 512], F32, tag="pv", name=f"pv{g}{cl}")
            # one bank holds all 4 rowsum accumulators (col-tiled):
            # rows 0:32 = cA h0, 32:64 = cA h1, 64:96 = cB h0, 96:128 = cB h1
            rs = psC.tile([128, 512], F32, tag="rsb", name=f"rs{g}")

            def epilogue(cl):
                dsb = tmp.tile([33, 512], BF16, tag="dsb", name="dsb")
                nc.vector.tensor_copy(out=dsb, in_=rs[64 * cl:64 * cl + 33, :])
                bc = psA.tile([128, 2, 512], F32, tag="big", name="bc")
                nc.tensor.matmul(bc[:, 0, :], ones33, dsb, start=True, stop=True)
                rcp = apool.tile([128, 512], F32, tag="rcp", name="rcp")
                nc.vector.reciprocal_approx_fast(rcp, bc[:, 0, :])
                att = apool.tile([128, 512], BF16, tag="att", name="att")
                nc.vector.tensor_mul(att, pv[cl], rcp)
                for j in range(4):
                    nc.sync.dma_start(
                        out=a2a_in[g][4 * cl + j, :, :],
                        in_=att[:, j * 128:(j + 1) * 128],
                    )

            def emit_pv(work):
                kt, cl, ts0, p = work
                first = kt == 0
                last = kt == 4 * (2 * c2 + cl) + 3
                blk = b * 16 + kt
                nc.tensor.matmul(
                    pv[cl][0:64, ts0:512], vsb[:, blk, 0:64], p[:, 0, ts0:512],
                    start=first, stop=last, tile_position=(0, 0),
                    skip_group_check=True,
                )
                nc.tensor.matmul(
                    pv[cl][64:128, ts0:512], vsb[:, blk, 64:128],
                    p[:, 1, ts0:512],
                    start=first, stop=last, tile_position=(0, 64),
                    skip_group_check=True,
                )
                ro = 64 * cl
                nc.tensor.matmul(
                    rs[ro:ro + 32, ts0:512], onesK, p[:, 0, ts0:512],
                    start=first, stop=last, tile_position=(0, ro),
                    skip_group_check=True,
                )
                nc.tensor.matmul(
                    rs[ro + 32:ro + 64, ts0:512], onesK, p[:, 1, ts0:512],
                    start=first, stop=last, tile_position=(0, ro + 32),
                    skip_group_check=True,
                )
                if last and not (defer_tail and cl == 1):
                    epilogue(cl)

            pairs = []
            for cl in range(2):
                for kt in range(4 * (2 * c2 + cl) + 4):
                    pairs.append((kt, cl))
            npairs = len(pairs)
            # evenly spread filler pop points across the pair loop
            pending = deque()
            fillers = sorted(fillers, key=lambda df: df[0])
            nf = len(fillers)
            spread = {min(npairs - 1, ((i + 1) * npairs) // (nf + 1))
                      for i in range(nf)} if nf else set()
            for npair, (kt, cl) in enumerate(pairs):
                kcol = b * T + kt * 128
                if True:
                    ch0 = c2 * 1024 + cl * 512
                    while fillers and fillers[0][0] <= npair:
                        fillers.pop(0)[1]()
                    if fillers and npair in spread:
                        fillers.pop(0)[1]()
                    ts0 = max(0, kt * 128 - ch0)
                    tcols = slice(b * T + ch0 + ts0, b * T + ch0 + 512)
                    ps_s = psA.tile([128, 2, 512], F32, tag="big", name="ps_s")
                    nc.tensor.matmul(
                        ps_s[:, 0, ts0:512], kt_sb[0:64, kcol:kcol + 128],
                        qt[0:64, tcols],
                        start=True, stop=True, tile_position=(0, 0),
                    )
                    nc.tensor.matmul(
                        ps_s[:, 1, ts0:512], kt_sb[64:128, kcol:kcol + 128],
                        qt[64:128, tcols],
                        start=True, stop=True, tile_position=(64, 0),
                    )
                    p = ppool.tile([128, 2, 512], BF16, tag="p", name="p")
                    nc.scalar.activation(
                        out=p[:, :, ts0:512], in_=ps_s[:, :, ts0:512],
                        func=mybir.ActivationFunctionType.Exp, scale=SCALE,
                    )
                    if kt * 128 >= ch0:  # diagonal block: causal 0/1 mask
                        nc.gpsimd.tensor_mul(
                            p[:, :, ts0:ts0 + 128], p[:, :, ts0:ts0 + 128],
                            tri_sb,
                        )
                    pending.append((kt, cl, ts0, p))
                    if len(pending) > 1:
                        emit_pv(pending.popleft())
            while pending:
                emit_pv(pending.popleft())
            for _, f in fillers:
                f()

            def tail():
                if defer_tail:
                    epilogue(1)
                # head-sharded -> token-sharded for this 1024-tok group
                if skip_collective:
                    nc.sync.dma_start(out=a2a_out[g][:], in_=a2a_in[g][:])
                else:
                    nc.gpsimd.collective_compute(
                        "AllToAll",
                        mybir.AluOpType.bypass,
                        replica_groups=[list(range(NCORES))],
                        ins=[a2a_in[g][:]],
                        outs=[a2a_out[g][:]],
                    )
            if defer_tail:
                return tail
            tail()

        # ---- emission schedule ----
        # PE executes strictly in emission order; fillers carry a due-pair
        # deadline so every tile is emitted before its first consumer.
        def P1(b, c, w_sb, dst):
            return lambda: qkv_piece_1(b, c, w_sb, dst)

        def PV_(b, c, jh):
            return lambda: qkv_piece_v(b, c, jh)

        qkv_piece_1(0, 0, wk, kt_sb)
        qkv_piece_1(0, 0, wq, qt)
        qkv_piece_v(0, 0, 0)
        t0_ = attn_group(0, 0, [
            (1, P1(0, 1, wq, qt)), (1, PV_(0, 0, 1)),
            (5, P1(0, 1, wk, kt_sb)), (7, PV_(0, 1, 0)), (8, PV_(0, 1, 1)),
            (9, P1(0, 2, wq, qt)), (10, P1(0, 2, wk, kt_sb)),
            (11, PV_(0, 2, 0)), (11, PV_(0, 2, 1)),
            (11, P1(0, 3, wq, qt)), (11, P1(0, 3, wk, kt_sb)),
            (11, PV_(0, 3, 0)), (11, PV_(0, 3, 1)),
        ], defer_tail=True)
        t1_ = attn_group(0, 1, [
            (0, t0_),
            (8, P1(1, 0, wk, kt_sb)), (10, PV_(1, 0, 0)),
            (12, P1(1, 0, wq, qt)), (14, PV_(1, 0, 1)),
            (16, P1(1, 1, wk, kt_sb)), (18, PV_(1, 1, 0)),
            (20, P1(1, 1, wq, qt)), (22, PV_(1, 1, 1)),
            (24, P1(1, 2, wq, qt)), (26, P1(1, 2, wk, kt_sb)),
        ], defer_tail=True)
        # big group (1,1) before (1,0): dense fillers; the final small group
        # runs with outproj fillers and a short exposed a2a
        tA_ = attn_group(1, 1, [
            (0, t1_),
            (7, PV_(1, 2, 0)), (9, PV_(1, 2, 1)), (9, P1(1, 3, wq, qt)),
            (20, P1(1, 3, wk, kt_sb)), (22, PV_(1, 3, 0)),
            (23, PV_(1, 3, 1)), (27, lambda: outproj(0)),
        ], defer_tail=True)
        tB_ = attn_group(1, 0, [(0, tA_), (6, lambda: outproj(1))],
                         defer_tail=True)
        outproj(3)
        tB_()
        outproj(2)

    nc.compile()
    return nc


_NC = None


def _get_nc():
    global _NC
    if _NC is None:
        _NC = build_nc()
    return _NC


def _rope_perm():
    # per-head 16-row stripes [x1 pairs 0:16 | x2 pairs 0:16 | x1 16:32 |
    # x2 16:32] so the rotate-half swap is a within-32 partition permute
    # (DVE stream_shuffle, mask (i+16)%32).
    p = []
    for h in range(2):
        o = h * DH
        p.extend(o + np.arange(0, 32, 2))   # evens of pairs 0:16
        p.extend(o + np.arange(1, 32, 2))   # odds of pairs 0:16
        p.extend(o + np.arange(32, 64, 2))  # evens of pairs 16:32
        p.extend(o + np.arange(33, 64, 2))  # odds
    return np.asarray(p)


def make_in_maps(inputs):
    x = np.asarray(inputs["in_features"], np.float32)
    Wq = np.asarray(inputs["Wq"], np.float32)
    Wk = np.asarray(inputs["Wk"], np.float32)
    Wv = np.asarray(inputs["Wv"], np.float32)
    Wo = np.asarray(inputs["Wo"], np.float32)
    pos = np.asarray(inputs["token_positions"]).astype(np.float32)[None, :]

    bf = ml_dtypes.bfloat16
    XT = np.ascontiguousarray(x.reshape(TOK, D).T).astype(bf)
    WoT = np.ascontiguousarray(Wo.T).astype(bf)
    perm = _rope_perm()

    in_maps = []
    for c in range(NCORES):
        rows = slice(c * 128, (c + 1) * 128)
        wq_c = Wq[rows][perm]
        wk_c = Wk[rows][perm]
        wv_c = Wv[rows]
        in_maps.append({
            "xt": XT,
            "wqt": np.ascontiguousarray(wq_c.T).astype(bf),
            "wkt": np.ascontiguousarray(wk_c.T).astype(bf),
            "wvt": np.ascontiguousarray(wv_c.T).astype(bf),
            "wot": WoT,
            "posf": pos,
        })
    return in_maps


def assemble_out(results):
    full = np.empty((TOK, D), np.float32)
    for c in range(NCORES):
        chunk = np.asarray(results[c]["out"])  # [512, D] out^T rows
        for g in range(NG):
            t0 = 1024 * g + 128 * c
            full[t0:t0 + 128] = chunk[g * 128:(g + 1) * 128]
    return full.reshape(B, T, D)


def run(inputs, **kwargs):
    nc = _get_nc()
    res = run_bass_kernel_spmd(
        nc, make_in_maps(inputs), core_ids=list(range(NCORES)), **kwargs
    )
    return assemble_out(res.results), res


def kernel(**inputs) -> np.ndarray:
    out, _ = run(inputs)
    return out


# ---------------------------------------------------------------------------
# Benchmark path: cached jitted executable so repeat executions can be timed
# without retracing/recompiling. Mirrors bass2jax.run_bass_via_pjrt.
# ---------------------------------------------------------------------------
_EXEC = {}


def _build_exec(kind="main"):
    if kind in _EXEC:
        return _EXEC[kind]
    import jax
    from jax.experimental.shard_map import shard_map
    from jax.sharding import Mesh, PartitionSpec

    import concourse.mybir as mybir
    from concourse import bass2jax

    nc = _get_nc() if kind == "main" else build_nc(dummy=True)
    bass2jax.install_neuronx_cc_hook()

    partition_name = nc.partition_id_tensor.name if nc.partition_id_tensor else None
    in_names, out_names, out_avals, zero_outs = [], [], [], []
    for alloc in nc.m.functions[0].allocations:
        if not isinstance(alloc, mybir.MemoryLocationSet):
            continue
        name = alloc.memorylocations[0].name
        if alloc.kind == "ExternalInput":
            if name != partition_name:
                in_names.append(name)
        elif alloc.kind == "ExternalOutput":
            out_names.append(name)
            shape = tuple(alloc.tensor_shape)
            dtype = mybir.dt.np(alloc.dtype)
            out_avals.append(jax.core.ShapedArray(shape, dtype))
            zero_outs.append(np.zeros(shape, dtype))
    n_params = len(in_names)
    all_names = list(in_names) + list(out_names)
    if partition_name is not None:
        all_names.append(partition_name)

    def _body(*args):
        outs = bass2jax._bass_exec_p.bind(
            *(list(args) + ([bass2jax.partition_id_tensor()]
                            if partition_name is not None else [])),
            out_avals=tuple(out_avals),
            in_names=tuple(all_names),
            out_names=tuple(out_names),
            lowering_input_output_aliases=(),
            sim_require_finite=True,
            sim_require_nnan=True,
            nc=nc,
        )
        return tuple(outs)

    devices = jax.devices()[:NCORES]
    mesh = Mesh(np.asarray(devices), ("core",))
    nspec = n_params + len(out_names)
    sharded = jax.jit(
        shard_map(
            _body,
            mesh=mesh,
            in_specs=(PartitionSpec("core"),) * nspec,
            out_specs=(PartitionSpec("core"),) * len(out_names),
            check_rep=False,
        ),
        keep_unused=True,
    )
    _EXEC[kind] = (sharded, in_names, out_names, zero_outs, mesh)
    return _EXEC[kind]


def _stage_args(inputs, exec_tuple):
    import jax
    from jax.sharding import NamedSharding, PartitionSpec

    sharded, in_names, out_names, zero_outs, mesh = exec_tuple
    in_maps = make_in_maps(inputs)
    sh = NamedSharding(mesh, PartitionSpec("core"))
    args = []
    for name in in_names:
        cat = np.concatenate([in_maps[c][name] for c in range(NCORES)], axis=0)
        args.append(jax.device_put(cat, sh))
    for z in zero_outs:
        cat = np.concatenate([z] * NCORES, axis=0)
        args.append(jax.device_put(cat, sh))
    return args


def _timed(fn, args, iters):
    import time

    import jax

    jax.block_until_ready(fn(*args))
    best = float("inf")
    for _ in range(iters):
        t0 = time.perf_counter()
        outs = fn(*args)
        jax.block_until_ready(outs)
        best = min(best, time.perf_counter() - t0)
    return outs, best


def run_bench(inputs, iters=10):
    """Returns (output, est_exec_seconds, t_full, t_dummy)."""
    e1 = _build_exec("main")
    args = _stage_args(inputs, e1)
    outs, t_full = _timed(e1[0], args, iters)
    ed = _build_exec("dummy")
    _timed(ed[0], args, iters)
    _, t_dummy = _timed(ed[0], args, iters)

    _, in_names, out_names, zero_outs, mesh = e1
    results = []
    for c in range(NCORES):
        m = {}
        for i, name in enumerate(out_names):
            arr = np.asarray(outs[i])
            per = arr.shape[0] // NCORES
            m[name] = arr[c * per:(c + 1) * per]
        results.append(m)
    return assemble_out(results), max(t_full - t_dummy, 0.0), t_full, t_dummy
